# revision 6
# baseline (speedup 1.0000x reference)
"""TT-adapter linear kernel for TRN2, data-parallel over batch on 8 NeuronCores.

Math: out = x @ W.T + b + ALPHA * TT(x).  TT is linear in x, so the module
collapses to a single matmul with a merged weight folded on host:

    Wc = W + ALPHA * T          (T = TT-matrix reconstruction, 1024x1024)
    out = x @ Wc.T + b

Mixed-precision hybrid: the rel-err gate (2e-2) leaves room to run PART of
the contraction in fp8.  Contraction rows 768..1023 (2 of 8 k-tiles) use
e4m3 for both operands via ONE DoubleRow matmul per output tile (virtual
256-deep contraction; the moving operand streams 2 fp8/cycle, so the DR MM
costs the same ~216 ns slot as ONE bf16 MM while covering TWO k-tiles).
Rows 0..767 stay bf16.  Measured rel err 1.62e-2 (vs 2.6e-3 all-bf16,
3.2e-2 all-fp8).  Per group: 7 MM slots instead of 8 -> PE stream ~48.8 us
instead of 55.4.

e4m3 subnormal trap: Wc entries (~0.012 std) sit below e4m3's min normal
2^-6, so weights are scaled x16 host-side.  To keep ONE PSUM accumulation,
the bf16 weights are ALSO scaled x16 (exact in bf16 -- exponent shift) and
the eviction descales: ACTIVATE Identity out = psum * (1/16) + bias.

Measured DMA behavior that shapes the schedule: in-order HWDGE queues,
~230 GB/s cold / ~430 GB/s warm streaming, ~0.45 us fixed cost per DMA +
~0.65 us ISSUE cost on the issuing engine, completion sem reaches +16 only
0.3-1.4 us after data lands, ~0.8 us first-use queue spin-up.  The first
wx row (the PE's first gate) is split by PARTITION HALVES across the SP
and ACT queues so two cold queues stream it concurrently (also pre-warms
ACT's queue for the tail out-DMA).  The PE must stay continuously busy
from the preamble until real data arrives, else the HAM clock-gate
re-throttles.

Host layouts (per core, P=128 partitions, contraction dim on partitions):
    wxs0 bf16 [6, 128, 1544] [d, p, 0:512]      = x[b, 0:512, 128d+p]
                             [0, p, 512+oo]     = b[128oo+p]
                             [d, p, 520:1544]   = 16*Wc[:, 128d+p] (all o)
    wx8  e4m3 [128, 2, 1536] [p, i, 0:1024]     = 16*Wc[:, 768+128i+p]
                             [p, i, 1024:1536]  = x[b, 0:512, 768+128i+p]
    xs   bf16 [128, 3, 6, 512] [p, sc-1, d, j]  = x[b, 512sc+j, 128d+p]
    xs8  e4m3 [128, 3, 2, 512] [p, sc-1, i, j]  = x[b, 512sc+j, 768+128i+p]
    out  bf16 [8, 128, 2048]   [oo, p, s]       = result[b, s, 128oo+p]

Schedule per core (group idx = 8*sc + o; bank = o):
  SP:  wx0 lower partition half; wx d=1..5; wx8; xs8 (all sc); xs sc=1;
       xs sc=2,3; then out-DMAs for groups 0..30 + 31a gated on evictions;
       final wait on slot sems.
  ACT: wx0 upper partition half FIRST (warms this queue), dummy 8-col
       activate (hoists the lazy ACT_TABLE_LOAD), 33 evictions
       (psum * 1/16 + bias -> bf16), ships group 31b (last 128 cols) from
       its own queue on the critical tail.
  PE:  12 HAM-warm-up matmuls, then phase 1 = sc=0 strip d-outer staircase
       across all 8 PSUM banks (6 bf16 rounds + 1 DR round); phase 2 =
       sc=1..3 strips, d-inner per group (6 bf16 + DR, stop on DR), each
       group's gate waits hoisted before the previous group's last MM.
       The last group (o=7, sc=3) is split 384+128 cols (the 128-col part
       lands in bank 0) so the final eviction+DMA chain is ~4x smaller.
"""

import numpy as np
import ml_dtypes
from contextlib import ExitStack

import concourse.bass as bass  # noqa: F401
import concourse.mybir as mybir
from concourse import bacc
from concourse.bass_utils import run_bass_kernel_spmd

ALPHA = 16.0
B, S, D = 8, 2048, 1024
P = 128
DO = D // P          # 8 contraction tiles
DB = 6               # bf16 contraction tiles (d=0..5); d=6,7 ride fp8 DR
OO = D // P          # 8 output tiles
SCH = 512
NS = S // SCH        # 4 s-chunks
NG = OO * NS         # 32 logical groups
NBANK = 8
NSLOT = 4
WSCALE = 16.0        # weight scale (both bf16 and fp8); evict descales
XOFF = 0
BOFF = SCH           # bias at cols 512..520 of wxs0 row 0
WOFF = SCH + 8
WXC = SCH + 8 + D    # 1544 cols per bf16 d-row: x0 | bias | w
W8C = D + SCH        # 1536 cols per fp8 half: w | x0
LASTA = 384          # last group's first sub-chunk width
HP = P // 2          # partition half

_NC = None

# Work units: (o, sc, bank, col0, ncol, slot, sl_off) for phase 2 + tail.
# Groups 8..30 are full 512-col groups; the last logical group (o=7,sc=3)
# is split into 31a (384 cols, bank 7) and 31b (128 cols, bank 0 -- its
# previous user, group 24, is evicted long before).
def _units():
    us = []
    for g in range(NBANK, NG - 1):
        us.append(dict(o=g % OO, sc=g // OO, bank=g % OO, c0=0, n=SCH,
                       slot=g % NSLOT, ev_req=g - NBANK + 1))
    us.append(dict(o=7, sc=3, bank=7, c0=0, n=LASTA, slot=3, ev_req=24))
    us.append(dict(o=7, sc=3, bank=0, c0=LASTA, n=SCH - LASTA, slot=0,
                   ev_req=25))
    return us


def _build_nc():
    nc = bacc.Bacc("TRN2", target_bir_lowering=False, debug=False)
    wxs0 = nc.declare_dram_parameter("wxs0", [DB, P, WXC], mybir.dt.bfloat16, isOutput=False)
    wx8 = nc.declare_dram_parameter("wx8", [P, 2, W8C], mybir.dt.float8e4, isOutput=False)
    xs = nc.declare_dram_parameter("xs", [P, NS - 1, DB, SCH], mybir.dt.bfloat16, isOutput=False)
    xs8 = nc.declare_dram_parameter("xs8", [P, NS - 1, 2, SCH], mybir.dt.float8e4, isOutput=False)
    out = nc.declare_dram_parameter("out", [OO, P, S], mybir.dt.bfloat16, isOutput=True)

    units = _units()

    with ExitStack() as ctx:
        block = ctx.enter_context(nc.Block(no_gpsimd_drain=True))
        s_wx0 = ctx.enter_context(nc.semaphore("s_wx0"))   # both wx0 halves
        s_wx = [ctx.enter_context(nc.semaphore(f"s_wx{d}")) for d in range(1, DB)]
        s_w8 = ctx.enter_context(nc.semaphore("s_w8"))     # wx8 (DR weights + x0)
        s_x8 = ctx.enter_context(nc.semaphore("s_x8"))     # xs8 (DR x, sc=1..3)
        s_x1 = ctx.enter_context(nc.semaphore("s_x1"))     # xs sc=1
        s_x23 = ctx.enter_context(nc.semaphore("s_x23"))   # xs sc=2,3
        s_mm = ctx.enter_context(nc.semaphore("s_mm"))
        s_ev = ctx.enter_context(nc.semaphore("s_ev"))
        s_slot = [ctx.enter_context(nc.semaphore(f"s_slot{k}")) for k in range(NSLOT)]

        wx_sb = ctx.enter_context(nc.sbuf_tensor("wx_sb", [P, DB, WXC], mybir.dt.bfloat16))
        wx8_sb = ctx.enter_context(nc.sbuf_tensor("wx8_sb", [P, 2, W8C], mybir.dt.float8e4))
        xb_sb = ctx.enter_context(nc.sbuf_tensor("xb_sb", [P, NS - 1, DB, SCH], mybir.dt.bfloat16))
        xs8_sb = ctx.enter_context(nc.sbuf_tensor("xs8_sb", [P, NS - 1, 2, SCH], mybir.dt.float8e4))
        ot_sb = ctx.enter_context(nc.sbuf_tensor("ot_sb", [P, NSLOT, SCH], mybir.dt.bfloat16))
        ps = [ctx.enter_context(nc.psum_tensor(f"ps{b}", [P, SCH], mybir.dt.float32))
              for b in range(NBANK)]

        def wsl(o, d):
            return wx_sb[:, d, WOFF + o * P:WOFF + (o + 1) * P]

        def xsl(sc, d, c0=0, n=SCH):
            if sc == 0:
                return wx_sb[:, d, XOFF + c0:XOFF + c0 + n]
            return xb_sb[:, sc - 1, d, c0:c0 + n]

        def dr_w(o):
            return wx8_sb[:, 0:2, o * P:(o + 1) * P]

        def dr_x(sc, c0=0, n=SCH):
            if sc == 0:
                return wx8_sb[:, 0:2, D + c0:D + c0 + n]
            return xs8_sb[:, sc - 1, 0:2, c0:c0 + n]

        def bias_ap(o):
            return wx_sb[:, 0, BOFF + o:BOFF + o + 1]

        @block.sync
        def _(sync: bass.BassEngine):
            # strict need-order, one DMA per staircase step; wx0's upper
            # partition half streams concurrently from the ACT queue
            sync.dma_start(out=wx_sb[0:HP, 0, :], in_=wxs0[0, 0:HP, :]).then_inc(s_wx0, 16)
            for d in range(1, DB):
                sync.dma_start(out=wx_sb[:, d, :], in_=wxs0[d]).then_inc(s_wx[d - 1], 16)
            sync.dma_start(out=wx8_sb[:, :, :], in_=wx8[:, :, :]).then_inc(s_w8, 16)
            sync.dma_start(out=xs8_sb[:, :, :, :], in_=xs8[:, :, :, :]).then_inc(s_x8, 16)
            sync.dma_start(out=xb_sb[:, 0, :, :], in_=xs[:, 0, :, :]).then_inc(s_x1, 16)
            sync.dma_start(out=xb_sb[:, 1:, :, :], in_=xs[:, 1:, :, :]).then_inc(s_x23, 16)
            for j in range(NG - 1):
                o, sc = j % OO, j // OO
                sync.wait_ge(s_ev, j + 1)
                sync.dma_start(
                    out=out[o, :, sc * SCH:(sc + 1) * SCH],
                    in_=ot_sb[:, j % NSLOT, :],
                ).then_inc(s_slot[j % NSLOT], 16)
            # 31a: first 384 cols of the last group
            sync.wait_ge(s_ev, NG)
            sync.dma_start(
                out=out[7, :, 3 * SCH:3 * SCH + LASTA],
                in_=ot_sb[:, 3, 0:LASTA],
            ).then_inc(s_slot[3], 16)
            sync.wait_ge(s_slot[0], 16 * 9)   # 8 via SP + 31b via ACT
            for k in range(1, NSLOT):
                sync.wait_ge(s_slot[k], 16 * 8)

        @block.tensor
        def _(tensor: bass.BassEngine):
            # HAM warm-up: continuous dummy matmuls so the PE clock-gate
            # reaches 8/8 and STAYS there until the first real gate passes
            # (~10us); results discarded (bank 0 restarts, start=True).
            for _ in range(12):
                tensor.matmul(
                    ps[0][:, 0:256],
                    wx_sb[:, 0, 0:P],
                    wx_sb[:, 1, 0:256],
                    start=True,
                    stop=True,
                )
            # phase 1: sc=0 strip, d-outer staircase over banks 0..7 (=o).
            # The NEXT step's gate wait is hoisted before each step's last
            # MM so the NX resolves it while the PE streams.
            tensor.wait_ge(s_wx0, 32)
            for d in range(DB):
                for o in range(OO):
                    if o == OO - 1:
                        if d < DB - 1:
                            tensor.wait_ge(s_wx[d], 16)
                        else:
                            tensor.wait_ge(s_w8, 16)
                    tensor.matmul(ps[o][:, :], wsl(o, d), xsl(0, d),
                                  start=(d == 0), stop=False)
            # DR round closes every sc=0 group in group order 0..7 so the
            # s_mm incs arrive in the order the evictions expect.
            for o in range(OO):
                if o == OO - 1:
                    # phase-2 g=8 gates, hoisted (already satisfied by now)
                    tensor.wait_ge(s_x1, 16)
                    tensor.wait_ge(s_x8, 16)
                    tensor.wait_ge(s_ev, 1)
                tensor.matmul(ps[o][:, :], dr_w(o), dr_x(0), start=False,
                              stop=True,
                              perf_mode=mybir.MatmulPerfMode.DoubleRow,
                              ).then_inc(s_mm, 1)
            # phase 2: sc=1..3 strips, d-inner per unit.  A unit's gate
            # waits are emitted before the previous unit's LAST matmul.
            for k, u in enumerate(units):
                # psum region always starts at col 0 of the unit's bank
                # (31b re-uses bank 0 cols 0:128 while its rhs cols are
                # 384:512 of the s-chunk)
                pdst = ps[u['bank']][:, 0:u['n']]
                for d in range(DB):
                    tensor.matmul(pdst, wsl(u['o'], d),
                                  xsl(u['sc'], d, u['c0'], u['n']),
                                  start=(d == 0), stop=False)
                if k + 1 < len(units):
                    nu = units[k + 1]
                    if nu['sc'] == 2 and nu['o'] == 0 and nu['c0'] == 0:
                        tensor.wait_ge(s_x23, 16)
                    tensor.wait_ge(s_ev, nu['ev_req'])
                tensor.matmul(pdst, dr_w(u['o']), dr_x(u['sc'], u['c0'], u['n']),
                              start=False, stop=True,
                              perf_mode=mybir.MatmulPerfMode.DoubleRow,
                              ).then_inc(s_mm, 1)

        @block.scalar
        def _(scalar: bass.BassEngine):
            # wx0 upper partition half: streams concurrently with the SP
            # queue's lower half AND warms this queue for the tail out-DMA.
            scalar.dma_start(out=wx_sb[HP:P, 0, :], in_=wxs0[0, HP:P, :]).then_inc(s_wx0, 16)
            # dummy 8-col activate: pulls the lazy ACT_TABLE_LOAD into the
            # preamble window.  Reads garbage; slot 0 is fully overwritten
            # by eviction 0 before any out-DMA reads it.
            scalar.activation(ot_sb[:, 0, 0:8], ot_sb[:, 1, 0:8],
                              mybir.ActivationFunctionType.Identity,
                              bias=bias_ap(0), scale=1.0 / WSCALE)
            # evictions j=0..30 (full groups), 31 (=31a), 32 (=31b)
            for j in range(NG + 1):
                if j < NG - 1:
                    o, bank, slot = j % OO, j % OO, j % NSLOT
                    c0, n, sl_need = 0, SCH, 16 * (j // NSLOT) if j >= NSLOT else 0
                elif j == NG - 1:
                    o, bank, slot, c0, n = 7, 7, 3, 0, LASTA
                    sl_need = 16 * 7
                else:
                    o, bank, slot, c0, n = 7, 0, 0, 0, SCH - LASTA
                    sl_need = 16 * 8
                if j == 0:
                    scalar.wait_ge(s_wx0, 32)   # bias rides in wx0
                scalar.wait_ge(s_mm, j + 1)
                if sl_need:
                    scalar.wait_ge(s_slot[slot], sl_need)
                scalar.activation(
                    ot_sb[:, slot, c0:c0 + n], ps[bank][:, c0:c0 + n],
                    mybir.ActivationFunctionType.Identity,
                    bias=bias_ap(o), scale=1.0 / WSCALE,
                ).then_inc(s_ev, 1)
            # 31b ships from ACT (own, pre-warmed queue): skips the SP
            # semaphore hop on the critical tail and is only 128 cols
            scalar.dma_start(
                out=out[7, :, 3 * SCH + LASTA:S],
                in_=ot_sb[:, 0, 0:SCH - LASTA],
            ).then_inc(s_slot[0], 16)

    nc.compile()
    return nc


def _get_nc():
    global _NC
    if _NC is None:
        _NC = _build_nc()
    return _NC


def _merged_weight_T(W, b, core0, core1, core2, core3, core4, core5):
    f8 = np.float64
    A = core0[0].astype(f8)
    Bm = np.einsum('ap,pbq->abq', A, core1.astype(f8))
    C = np.einsum('abq,qcr->abcr', Bm, core2.astype(f8))
    Phi = C.transpose(2, 1, 0, 3).reshape(D, 8)
    Dn = np.einsum('paq,qbr->pabr', core3.astype(f8), core4.astype(f8))
    E = np.einsum('pabq,qc->pabc', Dn, core5[:, :, 0].astype(f8))
    Psi = E.reshape(8, D)
    WcT = W.T.astype(f8) + ALPHA * (Phi @ Psi)
    return WcT * WSCALE


def _prep_in_maps(x, W, b, core0, core1, core2, core3, core4, core5):
    bf = ml_dtypes.bfloat16
    e4 = ml_dtypes.float8_e4m3
    WcTs = _merged_weight_T(W, b, core0, core1, core2, core3, core4, core5)
    wt16 = WcTs[:DB * P].reshape(DB, P, D).astype(bf)       # [d, p, o-cols]
    w8h = WcTs[DB * P:].astype(e4).reshape(2, P, D)          # [i, p, o-cols]
    bias_pad = np.zeros((DB, P, 8), dtype=bf)
    bias_pad[0] = b.reshape(OO, P).T.astype(bf)
    in_maps = []
    for bb in range(B):
        xt = x[bb].T.reshape(DO, P, NS, SCH)                 # [d, p, sc, j]
        x0 = xt[:DB, :, 0, :].astype(bf)
        wxs0 = np.ascontiguousarray(
            np.concatenate([x0, bias_pad, wt16], axis=2))
        x08 = xt[DB:, :, 0, :].astype(e4)                    # [i, p, j]
        wx8 = np.ascontiguousarray(
            np.concatenate([w8h, x08], axis=2).transpose(1, 0, 2))
        xsb = np.ascontiguousarray(
            xt[:DB, :, 1:, :].transpose(1, 2, 0, 3)          # [p, sc-1, d, j]
        ).astype(bf)
        xs8 = np.ascontiguousarray(
            xt[DB:, :, 1:, :].transpose(1, 2, 0, 3)          # [p, sc-1, i, j]
        ).astype(e4)
        in_maps.append({"wxs0": wxs0, "wx8": wx8, "xs": xsb, "xs8": xs8})
    return in_maps


def _gather(results):
    outs = []
    for bb in range(B):
        o = np.asarray(results[bb]["out"]).astype(np.float32)
        outs.append(o.transpose(2, 0, 1).reshape(S, D))
    return np.ascontiguousarray(np.stack(outs))


def run(inputs, **spmd_kwargs):
    inputs = {k: np.asarray(v) for k, v in inputs.items()}
    in_maps = _prep_in_maps(**inputs)
    nc = _get_nc()
    res = run_bass_kernel_spmd(nc, in_maps, core_ids=list(range(B)), **spmd_kwargs)
    return _gather(res.results), res


def kernel(x, W, b, core0, core1, core2, core3, core4, core5):
    out, _ = run(dict(x=x, W=W, b=b, core0=core0, core1=core1, core2=core2,
                      core3=core3, core4=core4, core5=core5))
    return out


# revision 7
# speedup vs baseline: 1.0353x; 1.0353x over previous
"""TT-adapter linear kernel for TRN2, data-parallel over batch on 8 NeuronCores.

Math: out = x @ W.T + b + ALPHA * TT(x).  TT is linear in x, so the module
collapses to a single matmul with a merged weight folded on host:

    Wc = W + ALPHA * T          (T = TT-matrix reconstruction, 1024x1024)
    out = x @ Wc.T + b

Mixed-precision hybrid: the rel-err gate (2e-2) leaves room to run PART of
the contraction in fp8.  Contraction rows 768..1023 (2 of 8 k-tiles) use
e4m3 for both operands via ONE DoubleRow matmul per output tile (virtual
256-deep contraction; the moving operand streams 2 fp8/cycle, so the DR MM
costs the same ~216 ns slot as ONE bf16 MM while covering TWO k-tiles).
Rows 0..767 stay bf16.  Measured rel err 1.62e-2 (vs 2.6e-3 all-bf16,
3.2e-2 all-fp8).  Per group: 7 MM slots instead of 8 -> PE stream ~48.8 us
instead of 55.4.

e4m3 subnormal trap: Wc entries (~0.012 std) sit below e4m3's min normal
2^-6, so weights are scaled x16 host-side.  To keep ONE PSUM accumulation,
the bf16 weights are ALSO scaled x16 (exact in bf16 -- exponent shift) and
the eviction descales: ACTIVATE Identity out = psum * (1/16) + bias.

Measured DMA behavior that shapes the schedule: in-order HWDGE queues,
~230 GB/s cold / ~430 GB/s warm streaming, ~0.45 us fixed cost per DMA +
~0.65 us ISSUE cost on the issuing engine, completion sem reaches +16 only
0.3-1.4 us after data lands, ~0.8 us first-use queue spin-up.  The first
wx row (the PE's first gate) is split by PARTITION HALVES across the SP
and ACT queues so two cold queues stream it concurrently (also pre-warms
ACT's queue for the tail out-DMA).  The PE must stay continuously busy
from the preamble until real data arrives, else the HAM clock-gate
re-throttles.

Host layouts (per core, P=128 partitions, contraction dim on partitions):
    wxs0 bf16 [6, 128, 1544] [d, p, 0:512]      = x[b, 0:512, 128d+p]
                             [0, p, 512+oo]     = b[128oo+p]
                             [d, p, 520:1544]   = 16*Wc[:, 128d+p] (all o)
    wx8  e4m3 [128, 2, 1536] [p, i, 0:1024]     = 16*Wc[:, 768+128i+p]
                             [p, i, 1024:1536]  = x[b, 0:512, 768+128i+p]
    xs   bf16 [128, 3, 6, 512] [p, sc-1, d, j]  = x[b, 512sc+j, 128d+p]
    xs8  e4m3 [128, 3, 2, 512] [p, sc-1, i, j]  = x[b, 512sc+j, 768+128i+p]
    out  bf16 [8, 128, 2048]   [oo, p, s]       = result[b, s, 128oo+p]

Schedule per core (group idx = 8*sc + o; bank = o):
  SP:  wx0 lower partition half; wx d=1..5; wx8; xs8 (all sc); xs sc=1;
       xs sc=2,3; then out-DMAs for groups 0..30 + 31a gated on evictions;
       final wait on slot sems.
  ACT: wx0 upper partition half FIRST (warms this queue), dummy 8-col
       activate (hoists the lazy ACT_TABLE_LOAD), 33 evictions
       (psum * 1/16 + bias -> bf16), ships group 31b (last 128 cols) from
       its own queue on the critical tail.
  PE:  12 HAM-warm-up matmuls, then phase 1 = sc=0 strip d-outer staircase
       across all 8 PSUM banks (6 bf16 rounds + 1 DR round); phase 2 =
       sc=1..3 strips, d-inner per group (6 bf16 + DR, stop on DR), each
       group's gate waits hoisted before the previous group's last MM.
       The last group (o=7, sc=3) is split 384+128 cols (the 128-col part
       lands in bank 0) so the final eviction+DMA chain is ~4x smaller.
"""

import numpy as np
import ml_dtypes
from contextlib import ExitStack

import concourse.bass as bass  # noqa: F401
import concourse.mybir as mybir
from concourse import bacc
from concourse.bass_utils import run_bass_kernel_spmd

ALPHA = 16.0
B, S, D = 8, 2048, 1024
P = 128
DO = D // P          # 8 contraction tiles
DB = 6               # bf16 contraction tiles (d=0..5); d=6,7 ride fp8 DR
OO = D // P          # 8 output tiles
SCH = 512
NS = S // SCH        # 4 s-chunks
NG = OO * NS         # 32 logical groups
NBANK = 8
NSLOT = 4
WSCALE = 16.0        # weight scale (both bf16 and fp8); evict descales
XOFF = 0
BOFF = SCH           # bias at cols 512..520 of wxs0 row 0
WOFF = SCH + 8
WXC = SCH + 8 + D    # 1544 cols per bf16 d-row: x0 | bias | w
W8C = D + SCH        # 1536 cols per fp8 half: w | x0
LASTA = 384          # last group's first sub-chunk width
HP = P // 2          # partition half

_NC = None

# Work units: (o, sc, bank, col0, ncol, slot, sl_off) for phase 2 + tail.
# Groups 8..30 are full 512-col groups; the last logical group (o=7,sc=3)
# is split into 31a (384 cols, bank 7) and 31b (128 cols, bank 0 -- its
# previous user, group 24, is evicted long before).
def _units():
    us = []
    for g in range(NBANK, NG - 1):
        us.append(dict(o=g % OO, sc=g // OO, bank=g % OO, c0=0, n=SCH,
                       slot=g % NSLOT, ev_req=g - NBANK + 1))
    us.append(dict(o=7, sc=3, bank=7, c0=0, n=LASTA, slot=3, ev_req=24))
    us.append(dict(o=7, sc=3, bank=0, c0=LASTA, n=SCH - LASTA, slot=0,
                   ev_req=25))
    return us


def _build_nc():
    nc = bacc.Bacc("TRN2", target_bir_lowering=False, debug=False)
    wxs0 = nc.declare_dram_parameter("wxs0", [DB, P, WXC], mybir.dt.bfloat16, isOutput=False)
    wx8 = nc.declare_dram_parameter("wx8", [P, 2, W8C], mybir.dt.float8e4, isOutput=False)
    xs = nc.declare_dram_parameter("xs", [P, NS - 1, DB, SCH], mybir.dt.bfloat16, isOutput=False)
    xs8 = nc.declare_dram_parameter("xs8", [P, NS - 1, 2, SCH], mybir.dt.float8e4, isOutput=False)
    out = nc.declare_dram_parameter("out", [OO, P, S], mybir.dt.bfloat16, isOutput=True)

    units = _units()

    with ExitStack() as ctx:
        block = ctx.enter_context(nc.Block(no_gpsimd_drain=True))
        s_wx0 = ctx.enter_context(nc.semaphore("s_wx0"))   # both wx0 halves
        s_wx = [ctx.enter_context(nc.semaphore(f"s_wx{d}")) for d in range(1, DB)]
        s_w8 = ctx.enter_context(nc.semaphore("s_w8"))     # wx8 (DR weights + x0)
        s_x8 = ctx.enter_context(nc.semaphore("s_x8"))     # xs8 (DR x, sc=1..3)
        s_x1 = ctx.enter_context(nc.semaphore("s_x1"))     # xs sc=1
        s_x23 = ctx.enter_context(nc.semaphore("s_x23"))   # xs sc=2,3
        s_mm = ctx.enter_context(nc.semaphore("s_mm"))
        s_ev = ctx.enter_context(nc.semaphore("s_ev"))
        s_scr = ctx.enter_context(nc.semaphore("s_scr"))
        s_slot = [ctx.enter_context(nc.semaphore(f"s_slot{k}")) for k in range(NSLOT)]

        wx_sb = ctx.enter_context(nc.sbuf_tensor("wx_sb", [P, DB, WXC], mybir.dt.bfloat16))
        wx8_sb = ctx.enter_context(nc.sbuf_tensor("wx8_sb", [P, 2, W8C], mybir.dt.float8e4))
        xb_sb = ctx.enter_context(nc.sbuf_tensor("xb_sb", [P, NS - 1, DB, SCH], mybir.dt.bfloat16))
        xs8_sb = ctx.enter_context(nc.sbuf_tensor("xs8_sb", [P, NS - 1, 2, SCH], mybir.dt.float8e4))
        ot_sb = ctx.enter_context(nc.sbuf_tensor("ot_sb", [P, NSLOT, SCH], mybir.dt.bfloat16))
        scr_sb = ctx.enter_context(nc.sbuf_tensor("scr_sb", [P, 16], mybir.dt.bfloat16))
        ps = [ctx.enter_context(nc.psum_tensor(f"ps{b}", [P, SCH], mybir.dt.float32))
              for b in range(NBANK)]

        def wsl(o, d):
            return wx_sb[:, d, WOFF + o * P:WOFF + (o + 1) * P]

        def xsl(sc, d, c0=0, n=SCH):
            if sc == 0:
                return wx_sb[:, d, XOFF + c0:XOFF + c0 + n]
            return xb_sb[:, sc - 1, d, c0:c0 + n]

        def dr_w(o):
            return wx8_sb[:, 0:2, o * P:(o + 1) * P]

        def dr_x(sc, c0=0, n=SCH):
            if sc == 0:
                return wx8_sb[:, 0:2, D + c0:D + c0 + n]
            return xs8_sb[:, sc - 1, 0:2, c0:c0 + n]

        def bias_ap(o):
            return wx_sb[:, 0, BOFF + o:BOFF + o + 1]

        @block.sync
        def _(sync: bass.BassEngine):
            # strict need-order, one DMA per staircase step
            sync.dma_start(out=wx_sb[:, 0, :], in_=wxs0[0]).then_inc(s_wx0, 16)
            for d in range(1, DB):
                sync.dma_start(out=wx_sb[:, d, :], in_=wxs0[d]).then_inc(s_wx[d - 1], 16)
            sync.dma_start(out=wx8_sb[:, :, :], in_=wx8[:, :, :]).then_inc(s_w8, 16)
            sync.dma_start(out=xs8_sb[:, :, :, :], in_=xs8[:, :, :, :]).then_inc(s_x8, 16)
            sync.dma_start(out=xb_sb[:, 0, :, :], in_=xs[:, 0, :, :]).then_inc(s_x1, 16)
            sync.dma_start(out=xb_sb[:, 1:, :, :], in_=xs[:, 1:, :, :]).then_inc(s_x23, 16)
            for j in range(NG - 1):
                o, sc = j % OO, j // OO
                sync.wait_ge(s_ev, j + 1)
                sync.dma_start(
                    out=out[o, :, sc * SCH:(sc + 1) * SCH],
                    in_=ot_sb[:, j % NSLOT, :],
                ).then_inc(s_slot[j % NSLOT], 16)
            for k in range(NSLOT):
                sync.wait_ge(s_slot[k], 16 * (NG // NSLOT))

        @block.tensor
        def _(tensor: bass.BassEngine):
            # HAM warm-up: continuous dummy matmuls so the PE clock-gate
            # reaches 8/8 and STAYS there until the first real gate passes
            # (~10us); results discarded (bank 0 restarts, start=True).
            for _ in range(16):
                tensor.matmul(
                    ps[0][:, 0:256],
                    wx_sb[:, 0, 0:P],
                    wx_sb[:, 1, 0:256],
                    start=True,
                    stop=True,
                )
            # phase 1: sc=0 strip, d-outer staircase over banks 0..7 (=o).
            # The NEXT step's gate wait is hoisted before each step's last
            # MM so the NX resolves it while the PE streams.
            tensor.wait_ge(s_wx0, 16)
            for d in range(DB):
                for o in range(OO):
                    if o == OO - 1:
                        if d < DB - 1:
                            tensor.wait_ge(s_wx[d], 16)
                        else:
                            tensor.wait_ge(s_w8, 16)
                    tensor.matmul(ps[o][:, :], wsl(o, d), xsl(0, d),
                                  start=(d == 0), stop=False)
            # DR round closes every sc=0 group in group order 0..7 so the
            # s_mm incs arrive in the order the evictions expect.
            for o in range(OO):
                if o == OO - 1:
                    # phase-2 g=8 gates, hoisted (already satisfied by now)
                    tensor.wait_ge(s_x1, 16)
                    tensor.wait_ge(s_x8, 16)
                    tensor.wait_ge(s_ev, 1)
                tensor.matmul(ps[o][:, :], dr_w(o), dr_x(0), start=False,
                              stop=True,
                              perf_mode=mybir.MatmulPerfMode.DoubleRow,
                              ).then_inc(s_mm, 1)
            # phase 2: sc=1..3 strips, d-inner per unit.  A unit's gate
            # waits are emitted before the previous unit's LAST matmul.
            for g in range(NBANK, NG):
                o, sc = g % OO, g // OO
                for d in range(DB):
                    tensor.matmul(ps[o][:, :], wsl(o, d), xsl(sc, d),
                                  start=(d == 0), stop=False)
                if g + 1 < NG:
                    if (g + 1) % OO == 0 and (g + 1) // OO == 2:
                        tensor.wait_ge(s_x23, 16)
                    tensor.wait_ge(s_ev, g + 1 - NBANK + 1)
                tensor.matmul(ps[o][:, :], dr_w(o), dr_x(sc), start=False,
                              stop=True,
                              perf_mode=mybir.MatmulPerfMode.DoubleRow,
                              ).then_inc(s_mm, 1)

        @block.scalar
        def _(scalar: bass.BassEngine):
            # warm ACT's HWDGE queue (used for the final out-DMA); scr_sb
            # is a dedicated scratch no one else touches.
            scalar.dma_start(out=scr_sb[:, :], in_=wxs0[0, :, 0:16]).then_inc(s_scr, 16)
            # dummy 8-col activate: pulls the lazy ACT_TABLE_LOAD into the
            # preamble window.  Reads garbage; slot 0 is fully overwritten
            # by eviction 0 before any out-DMA reads it.
            scalar.activation(ot_sb[:, 0, 0:8], ot_sb[:, 1, 0:8],
                              mybir.ActivationFunctionType.Identity,
                              bias=bias_ap(0), scale=1.0 / WSCALE)
            for g in range(NG):
                o, sc = g % OO, g // OO
                if g == 0:
                    scalar.wait_ge(s_wx0, 16)   # bias rides in wx0
                scalar.wait_ge(s_mm, g + 1)
                if g >= NSLOT:
                    scalar.wait_ge(s_slot[g % NSLOT], 16 * (g // NSLOT))
                scalar.activation(
                    ot_sb[:, g % NSLOT, :], ps[o][:, :],
                    mybir.ActivationFunctionType.Identity,
                    bias=bias_ap(o), scale=1.0 / WSCALE,
                ).then_inc(s_ev, 1)
                if g == NG - 1:
                    # last output ships from ACT (own, pre-warmed queue):
                    # skips the SP semaphore hop on the critical tail
                    scalar.dma_start(
                        out=out[o, :, sc * SCH:(sc + 1) * SCH],
                        in_=ot_sb[:, g % NSLOT, :],
                    ).then_inc(s_slot[g % NSLOT], 16)

    nc.compile()
    return nc


def _get_nc():
    global _NC
    if _NC is None:
        _NC = _build_nc()
    return _NC


def _merged_weight_T(W, b, core0, core1, core2, core3, core4, core5):
    f8 = np.float64
    A = core0[0].astype(f8)
    Bm = np.einsum('ap,pbq->abq', A, core1.astype(f8))
    C = np.einsum('abq,qcr->abcr', Bm, core2.astype(f8))
    Phi = C.transpose(2, 1, 0, 3).reshape(D, 8)
    Dn = np.einsum('paq,qbr->pabr', core3.astype(f8), core4.astype(f8))
    E = np.einsum('pabq,qc->pabc', Dn, core5[:, :, 0].astype(f8))
    Psi = E.reshape(8, D)
    WcT = W.T.astype(f8) + ALPHA * (Phi @ Psi)
    return WcT * WSCALE


def _prep_in_maps(x, W, b, core0, core1, core2, core3, core4, core5):
    bf = ml_dtypes.bfloat16
    e4 = ml_dtypes.float8_e4m3
    WcTs = _merged_weight_T(W, b, core0, core1, core2, core3, core4, core5)
    wt16 = WcTs[:DB * P].reshape(DB, P, D).astype(bf)       # [d, p, o-cols]
    w8h = WcTs[DB * P:].astype(e4).reshape(2, P, D)          # [i, p, o-cols]
    bias_pad = np.zeros((DB, P, 8), dtype=bf)
    bias_pad[0] = b.reshape(OO, P).T.astype(bf)
    in_maps = []
    for bb in range(B):
        xt = x[bb].T.reshape(DO, P, NS, SCH)                 # [d, p, sc, j]
        x0 = xt[:DB, :, 0, :].astype(bf)
        wxs0 = np.ascontiguousarray(
            np.concatenate([x0, bias_pad, wt16], axis=2))
        x08 = xt[DB:, :, 0, :].astype(e4)                    # [i, p, j]
        wx8 = np.ascontiguousarray(
            np.concatenate([w8h, x08], axis=2).transpose(1, 0, 2))
        xsb = np.ascontiguousarray(
            xt[:DB, :, 1:, :].transpose(1, 2, 0, 3)          # [p, sc-1, d, j]
        ).astype(bf)
        xs8 = np.ascontiguousarray(
            xt[DB:, :, 1:, :].transpose(1, 2, 0, 3)          # [p, sc-1, i, j]
        ).astype(e4)
        in_maps.append({"wxs0": wxs0, "wx8": wx8, "xs": xsb, "xs8": xs8})
    return in_maps


def _gather(results):
    outs = []
    for bb in range(B):
        o = np.asarray(results[bb]["out"]).astype(np.float32)
        outs.append(o.transpose(2, 0, 1).reshape(S, D))
    return np.ascontiguousarray(np.stack(outs))


def run(inputs, **spmd_kwargs):
    inputs = {k: np.asarray(v) for k, v in inputs.items()}
    in_maps = _prep_in_maps(**inputs)
    nc = _get_nc()
    res = run_bass_kernel_spmd(nc, in_maps, core_ids=list(range(B)), **spmd_kwargs)
    return _gather(res.results), res


def kernel(x, W, b, core0, core1, core2, core3, core4, core5):
    out, _ = run(dict(x=x, W=W, b=b, core0=core0, core1=core1, core2=core2,
                      core3=core3, core4=core4, core5=core5))
    return out
